# revision 6
# baseline (speedup 1.0000x reference)
"""Trainium2 Bass kernel for nn_BSQLinear (vq_codebook).

Reference computes:
    stacked = einsum('npl,plc->npc', vq_weight, w_dec) + b_dec     # (16384,4,256)
    w_flat  = stacked.transpose(1,0,2).reshape(4,-1)*(d_std+eps)+d_mean
    w_recon = w_flat.reshape(4,1024,4096).reshape(4096,4096)
    out     = x @ w_recon.T + bias                                  # (4,2048,4096)

Index algebra: with o = p*1024 + o_sub, i = n_sub*256 + c, n = o_sub*16 + n_sub:
    w_recon[o, i] = sum_l vq[n,p,l]*wdec'[p,l,c] + b'[p,c]
        wdec' = (d_std+eps)*w_dec,  b' = (d_std+eps)*b_dec + d_mean
so the 274-GFLOP GEMM factorizes through the rank-32 bottleneck:
    Y[t,p,ns,l] = sum_c x[t, ns*256+c] * wdec'[p,l,c]          (stage 1)
    out[t,o]    = sum_{ns,l} Y[t,p,ns,l]*vq[o_sub*16+ns,p,l]   (stage 2)
                  + S[t,p] + bias[o]
    S[t,p]      = sum_{ns,c} x[t,ns*256+c] * b'[p,c]           (S pass)
Total ~43 GFLOP instead of ~274 (the headroom=8 hint).

Sharding: data-parallel over the 8192 tokens -> 1024 tokens/core on 8 cores.
Each core DMAs 16 MiB x-slice + 8 MiB codebook in, 16 MiB out.
"""

import os
from contextlib import ExitStack

import numpy as np

import concourse.bacc as bacc
import concourse.bass as bass
import concourse.mybir as mybir
import concourse.tile as tile
from concourse.bass_utils import run_bass_kernel_spmd

P = 4
OUT_PER = 1024
IN_F = 4096
OUT_F = 4096
EPS = 1e-6
N_CORES = 8
T_TOTAL = 8192
TC = T_TOTAL // N_CORES  # 1024 tokens per core

F32 = mybir.dt.float32
# float32r: PE reads fp32 operands in "fast" mode -> 1 cycle/row at N>=256
# (vs 4 cycles/row for exact float32). Flip to F32 if hardware numerics of
# f32r fall outside the error budget.
MM_DT = mybir.dt.float32r if os.environ.get("BSQ_MM_DT", "f32r") == "f32r" else F32

LAST_RESULTS = None  # BassKernelResults from the most recent run (for test.py)


def _build_bass():
    nc = bacc.Bacc(None, target_bir_lowering=False)

    xs = nc.dram_tensor("xs", [TC, IN_F], MM_DT, kind="ExternalInput")
    wblk = nc.dram_tensor("wblk", [4, 4, 2, 128, 128], MM_DT, kind="ExternalInput")
    vq2 = nc.dram_tensor("vq2", [4, 4, 128, 1024], MM_DT, kind="ExternalInput")
    bpt = nc.dram_tensor("bpt", [2, 128, 4], MM_DT, kind="ExternalInput")
    ident = nc.dram_tensor("ident", [128, 128], MM_DT, kind="ExternalInput")
    ident4 = nc.dram_tensor("ident4", [4, 4], F32, kind="ExternalInput")
    biasrep = nc.dram_tensor("biasrep", [128, OUT_F], F32, kind="ExternalInput")
    out_d = nc.dram_tensor("out", [TC, OUT_F], F32, kind="ExternalOutput")

    with tile.TileContext(nc) as tc, ExitStack() as ctx:
        cpool = ctx.enter_context(tc.tile_pool(name="consts", bufs=1))
        ypool = ctx.enter_context(tc.tile_pool(name="y", bufs=1))
        xpool = ctx.enter_context(tc.tile_pool(name="x", bufs=6))
        xtpool = ctx.enter_context(tc.tile_pool(name="xt", bufs=10))
        opool = ctx.enter_context(tc.tile_pool(name="osb", bufs=3))
        spool = ctx.enter_context(tc.tile_pool(name="s", bufs=2))
        pp_t = ctx.enter_context(tc.tile_pool(name="ppt", bufs=2, space="PSUM"))
        pp_y = ctx.enter_context(tc.tile_pool(name="ppy", bufs=2, space="PSUM"))
        pp_o = ctx.enter_context(tc.tile_pool(name="ppo", bufs=2, space="PSUM"))
        pp_s = ctx.enter_context(tc.tile_pool(name="pps", bufs=1, space="PSUM"))
        pp_ss = ctx.enter_context(tc.tile_pool(name="ppss", bufs=1, space="PSUM"))

        # ---- resident constants ----
        wblk_sb = {}
        for p in range(4):
            for nsq in range(4):
                for ch in range(2):
                    t = cpool.tile([128, 128], MM_DT, tag=f"wb{p}{nsq}{ch}", name=f"wb{p}{nsq}{ch}")
                    nc.sync.dma_start(out=t[:, :], in_=wblk[p, nsq, ch])
                    wblk_sb[(p, nsq, ch)] = t
        vq2_sb = {}
        for p in range(4):
            for kc in range(4):
                t = cpool.tile([128, 1024], MM_DT, tag=f"vq{p}{kc}", name=f"vq{p}{kc}")
                nc.sync.dma_start(out=t[:, :], in_=vq2[p, kc])
                vq2_sb[(p, kc)] = t
        bpt_sb = []
        for ch in range(2):
            t = cpool.tile([128, 4], MM_DT, tag=f"bpt{ch}", name=f"bpt{ch}")
            nc.sync.dma_start(out=t[:, :], in_=bpt[ch])
            bpt_sb.append(t)
        ident_sb = cpool.tile([128, 128], MM_DT, tag="ident")
        nc.sync.dma_start(out=ident_sb[:, :], in_=ident[:, :])
        ident4_sb = cpool.tile([4, 4], F32, tag="ident4")
        nc.sync.dma_start(out=ident4_sb[:, :], in_=ident4[:, :])

        # ---- persistent Y and S2 tiles ----
        y_sb = {}
        for p in range(4):
            for kc in range(4):
                y_sb[(p, kc)] = ypool.tile([128, TC], MM_DT, tag=f"y{p}{kc}", name=f"y{p}{kc}")
        s2_sb = [spool.tile([128, 4], F32, tag=f"s2_{tc_i}", bufs=1, name=f"s2_{tc_i}") for tc_i in range(8)]

        for h in range(2):
            # S^T accumulator for this half: [p=4, t=512]
            ps_st = pp_s.tile([4, 512], F32, tag="st")
            for icq in range(4):  # i-quarter == kc; covers ns in [4*icq, 4*icq+4)
                xq = []
                for tsub in range(4):
                    t = xpool.tile([128, 1024], MM_DT, tag="xq", name=f"xq_{h}_{icq}_{tsub}")
                    nc.sync.dma_start(
                        out=t[:, :],
                        in_=xs[
                            h * 512 + tsub * 128 : h * 512 + (tsub + 1) * 128,
                            icq * 1024 : (icq + 1) * 1024,
                        ],
                    )
                    xq.append(t)

                # transpose x -> xt tiles [i-chunk(128), t(512)]
                xt = {}
                for nsq in range(4):
                    for ch in range(2):
                        pt = pp_t.tile([128, 512], MM_DT, tag="pt")
                        for tsub in range(4):
                            nc.tensor.matmul(
                                pt[:, tsub * 128 : (tsub + 1) * 128],
                                xq[tsub][:, nsq * 256 + ch * 128 : nsq * 256 + (ch + 1) * 128],
                                ident_sb[:, :],
                                is_transpose=True,
                                start=True,
                                stop=True,
                                skip_group_check=True,
                            )
                        xtile = xtpool.tile([128, 512], MM_DT, tag="xt", name=f"xt_{h}_{icq}_{nsq}_{ch}")
                        nc.any.tensor_copy(xtile[:, :], pt[:, :])
                        xt[(nsq, ch)] = xtile

                # stage 1: per p accumulate 8 matmuls -> Y[p][icq][:, h-half]
                for p in range(4):
                    py = pp_y.tile([128, 512], F32, tag="py")
                    for nsq in range(4):
                        for ch in range(2):
                            nc.tensor.matmul(
                                py[:, :],
                                wblk_sb[(p, nsq, ch)][:, :],
                                xt[(nsq, ch)][:, :],
                                start=(nsq == 0 and ch == 0),
                                stop=(nsq == 3 and ch == 1),
                            )
                    nc.any.tensor_copy(
                        y_sb[(p, icq)][:, h * 512 : (h + 1) * 512], py[:, :]
                    )

                # S pass: accumulate b'^T x over every i-chunk of this half
                for nsq in range(4):
                    for ch in range(2):
                        nc.tensor.matmul(
                            ps_st[:, :],
                            bpt_sb[ch][:, :],
                            xt[(nsq, ch)][:, :],
                            start=(icq == 0 and nsq == 0 and ch == 0),
                            stop=(icq == 3 and nsq == 3 and ch == 1),
                            skip_group_check=True,
                        )

            # finalize S for this half: evict, transpose [4,128]->[128,4] per t-chunk
            st_sb = spool.tile([4, 512], F32, tag="stsb")
            nc.any.tensor_copy(st_sb[:, :], ps_st[:, :])
            for tc4 in range(4):
                pss = pp_ss.tile([128, 4], F32, tag="pss")
                nc.tensor.matmul(
                    pss[:, :],
                    st_sb[:, tc4 * 128 : (tc4 + 1) * 128],
                    ident4_sb[:, :],
                    is_transpose=True,
                    start=True,
                    stop=True,
                    skip_group_check=True,
                )
                nc.any.tensor_copy(s2_sb[h * 4 + tc4][:, :], pss[:, :])

            # stage 2 for this half
            for p in range(4):
                for oh in range(2):
                    brs = opool.tile([128, 512], F32, tag="brs", bufs=2,
                                     name=f"brs_{h}_{p}_{oh}")
                    nc.sync.dma_start(
                        out=brs[:, :],
                        in_=biasrep[:, p * 1024 + oh * 512 : p * 1024 + (oh + 1) * 512],
                    )
                    for tm in range(4):
                        po = pp_o.tile([128, 512], F32, tag="po")
                        for kc in range(4):
                            nc.tensor.matmul(
                                po[:, :],
                                y_sb[(p, kc)][
                                    :, h * 512 + tm * 128 : h * 512 + (tm + 1) * 128
                                ],
                                vq2_sb[(p, kc)][:, oh * 512 : (oh + 1) * 512],
                                start=(kc == 0),
                                stop=(kc == 3),
                            )
                        osb = opool.tile([128, 512], F32, tag="osb")
                        # out = (psum + S[t,p]) + bias[o]
                        nc.vector.scalar_tensor_tensor(
                            osb[:, :],
                            po[:, :],
                            s2_sb[h * 4 + tm][:, p : p + 1],
                            brs[:, :],
                            op0=mybir.AluOpType.add,
                            op1=mybir.AluOpType.add,
                        )
                        nc.sync.dma_start(
                            out=out_d[
                                h * 512 + tm * 128 : h * 512 + (tm + 1) * 128,
                                p * 1024 + oh * 512 : p * 1024 + (oh + 1) * 512,
                            ],
                            in_=osb[:, :],
                        )

    nc.compile()
    return nc


_NC_CACHE = None


def _get_nc():
    global _NC_CACHE
    if _NC_CACHE is None:
        _NC_CACHE = _build_bass()
    return _NC_CACHE


def _host_prep(x, vq_weight, w_dec, b_dec, d_mean, d_std, bias):
    f4 = np.float32
    x2 = np.ascontiguousarray(x.reshape(T_TOTAL, IN_F), dtype=f4)
    scale = (d_std + EPS).astype(f4)  # (4,1)
    wdecp = (w_dec * scale[:, :, None]).astype(f4)  # (4,32,256)
    bp = (b_dec * scale + d_mean).astype(f4)  # (4,256)

    wdecT_p = np.ascontiguousarray(wdecp.transpose(0, 2, 1))  # (4,256,32) [p][c][l]
    wblk = np.zeros((4, 4, 2, 128, 128), dtype=f4)
    for nsq in range(4):
        for ch in range(2):
            wblk[:, nsq, ch, :, nsq * 32 : (nsq + 1) * 32] = wdecT_p[
                :, ch * 128 : (ch + 1) * 128, :
            ]

    # vq2[p][kc][(ns%4)*32+l][o_sub] = vq[o_sub*16+ns, p, l]
    vq2 = np.ascontiguousarray(
        vq_weight.reshape(1024, 16, 4, 32).transpose(2, 1, 3, 0).reshape(4, 4, 128, 1024),
        dtype=f4,
    )
    bpt = np.ascontiguousarray(bp.T.reshape(2, 128, 4), dtype=f4)  # [ch][cc][p]
    identity = np.eye(128, dtype=f4)
    identity4 = np.eye(4, dtype=f4)
    biasrep = np.ascontiguousarray(
        np.broadcast_to(bias.astype(f4), (128, OUT_F))
    )
    return x2, wblk, vq2, bpt, identity, identity4, biasrep


def kernel(x, vq_weight, w_dec, b_dec, d_mean, d_std, bias):
    global LAST_RESULTS
    x2, wblk, vq2, bpt, identity, identity4, biasrep = _host_prep(
        x, vq_weight, w_dec, b_dec, d_mean, d_std, bias
    )
    nc = _get_nc()
    in_maps = []
    for k in range(N_CORES):
        in_maps.append(
            {
                "xs": np.ascontiguousarray(x2[k * TC : (k + 1) * TC]),
                "wblk": wblk,
                "vq2": vq2,
                "bpt": bpt,
                "ident": identity,
                "ident4": identity4,
                "biasrep": biasrep,
            }
        )
    trace = os.environ.get("BSQ_TRACE", "0") == "1"
    res = run_bass_kernel_spmd(nc, in_maps, list(range(N_CORES)), trace=trace)
    LAST_RESULTS = res
    out = np.concatenate([res.results[k]["out"] for k in range(N_CORES)], axis=0)
    return out.reshape(4, 2048, OUT_F).astype(np.float32)
